# revision 9
# baseline (speedup 1.0000x reference)
"""Trainium2 Bass kernel for nn_DocLSTM_79018808312396 (GNN message passing).

Strategy: the dominant dense compute (the per-layer GAT feature transforms
x @ W for the 30000-node body graph, ~13.8 GFLOP total) runs as a Bass/Tile
SPMD kernel sharded node-parallel across the 8 NeuronCores (3840 rows/core,
one compiled NEFF reused for all three body layers via zero-padding to a
fixed [384-K, 256-N] shape). The irregular per-edge segment-softmax
bookkeeping, the tiny 64-node head graph, and the attention/pooling tail run
replicated on host (they are index-manipulation / sub-1% of the FLOPs).
"""

import numpy as np

N_BODY = 30000
M_PAD = 30720          # 30000 padded to 8 cores * 3840 (30 chunks of 128)
M_CORE = 3840
K_PAD = 384            # max fin (300) padded to 3 chunks of 128
N_PAD = 256            # max GAT_HEADS*fout (2*128)
N_CORES = 8

_NC_CACHE = {}


def _build_mm_nc():
    """out[m, n] = sum_k xT[k, m] * w[k, n]  (per core: [3840,384]@[384,256])"""
    import concourse.mybir as mybir
    import concourse.tile as tile
    from concourse import bacc

    nc = bacc.Bacc("TRN2", target_bir_lowering=False, debug=False,
                   num_devices=N_CORES)
    # Host pre-tiles x.T into contiguous [128,128] blocks: block (m, k) holds
    # xT[k*128:(k+1)*128, m*128:(m+1)*128] at rows (m*n_k + k)*128.
    xT = nc.dram_tensor(
        "xT", [(M_CORE // 128) * (K_PAD // 128) * 128, 128],
        mybir.dt.float32, kind="ExternalInput").ap()
    w = nc.dram_tensor("w", [K_PAD, N_PAD], mybir.dt.float32,
                       kind="ExternalInput").ap()
    out = nc.dram_tensor("out", [M_CORE, N_PAD], mybir.dt.float32,
                         kind="ExternalOutput").ap()

    n_m = M_CORE // 128   # 30 node chunks
    n_k = K_PAD // 128    # 3 contraction chunks

    with tile.TileContext(nc) as tc:
        with (
            tc.tile_pool(name="wpool", bufs=1) as wpool,
            tc.tile_pool(name="sb", bufs=4) as sb,
            tc.tile_pool(name="ob", bufs=3) as ob,
            tc.tile_pool(name="ps", bufs=2, space="PSUM") as ps,
        ):
            w_tiles = []
            for k in range(n_k):
                wt = wpool.tile([128, N_PAD], mybir.dt.float32, tag=f"w{k}")
                nc.gpsimd.dma_start(out=wt[:], in_=w[k * 128:(k + 1) * 128, :])
                w_tiles.append(wt)

            for m in range(n_m):
                psum = ps.tile([128, N_PAD], mybir.dt.float32)
                for k in range(n_k):
                    xt = sb.tile([128, 128], mybir.dt.float32, tag="xt")
                    blk = (m * n_k + k) * 128
                    nc.gpsimd.dma_start(out=xt[:], in_=xT[blk:blk + 128, :])
                    nc.tensor.matmul(
                        out=psum[:],
                        lhsT=xt[:],
                        rhs=w_tiles[k][:],
                        start=(k == 0),
                        stop=(k == n_k - 1),
                    )
                o = ob.tile([128, N_PAD], mybir.dt.float32, tag="o")
                nc.vector.tensor_copy(out=o[:], in_=psum[:])
                nc.sync.dma_start(out=out[m * 128:(m + 1) * 128, :], in_=o[:])
    nc.compile()
    return nc


def _device_mm(x, W):
    """x [30000, fin] @ W [fin, fout] -> [30000, fout] on 8 NeuronCores."""
    from concourse.bass_utils import run_bass_kernel_spmd

    fin, fout = W.shape
    if "nc" not in _NC_CACHE:
        _NC_CACHE["nc"] = _build_mm_nc()
    nc = _NC_CACHE["nc"]

    xTp = np.zeros((K_PAD, M_PAD), np.float32)
    xTp[:fin, :N_BODY] = np.asarray(x, np.float32).T
    wp = np.zeros((K_PAD, N_PAD), np.float32)
    wp[:fin, :fout] = np.asarray(W, np.float32)

    n_k = K_PAD // 128
    n_m = M_CORE // 128
    in_maps = []
    for c in range(N_CORES):
        sh = xTp[:, c * M_CORE:(c + 1) * M_CORE]          # [K_PAD, M_CORE]
        # -> [n_m*n_k*128, 128]: block (m, k) = sh[k*128:, m*128:]
        t = (sh.reshape(n_k, 128, n_m, 128)
               .transpose(2, 0, 1, 3)
               .reshape(n_m * n_k * 128, 128))
        in_maps.append({"xT": np.ascontiguousarray(t), "w": wp})
    res = run_bass_kernel_spmd(nc, in_maps, list(range(N_CORES))).results
    out = np.concatenate([np.asarray(res[c]["out"]) for c in range(N_CORES)],
                         axis=0)
    return out[:N_BODY, :fout]


def _leaky(x):
    return np.where(x >= 0, x, np.float32(0.2) * x)


def _gat(x, ei, p, xp_fn):
    """GATConv matching the reference; xp_fn computes x @ W."""
    x = np.asarray(x, np.float32)
    N = x.shape[0]
    W = np.asarray(p["W"], np.float32)
    a_src = np.asarray(p["a_src"], np.float32)
    a_dst = np.asarray(p["a_dst"], np.float32)
    b = np.asarray(p["b"], np.float32)
    H, F = a_src.shape

    xp = xp_fn(x, W).reshape(N, H, F)
    loop = np.arange(N, dtype=np.asarray(ei).dtype)
    src = np.concatenate([np.asarray(ei[0]), loop])
    dst = np.concatenate([np.asarray(ei[1]), loop])

    a_s = (xp * a_src).sum(-1)      # [N, H]
    a_d = (xp * a_dst).sum(-1)
    alpha = _leaky(a_s[src] + a_d[dst])   # [E, H]

    order = np.argsort(dst, kind="stable")
    ds_, ss, al = dst[order], src[order], alpha[order]
    starts = np.searchsorted(ds_, np.arange(N))  # every node has a self loop

    m = np.maximum.reduceat(al, starts, axis=0)          # [N, H]
    e = np.exp(al - m[ds_])
    denom = np.add.reduceat(e, starts, axis=0)
    coef = e / np.maximum(denom[ds_], np.float32(1e-16))

    msg = coef[:, :, None].astype(np.float32) * xp[ss]   # [E, H, F]
    out = np.add.reduceat(msg, starts, axis=0)           # [N, H, F]
    return out.mean(1) + b


def _softmax(x, axis=-1):
    m = x.max(axis=axis, keepdims=True)
    e = np.exp(x - m)
    return e / e.sum(axis=axis, keepdims=True)


def _mha(q, k, v, indicator, p):
    B, E = q.shape[0], q.shape[-1]
    NH = 4
    D = E // NH

    def split(x, w, bb):
        y = x @ np.asarray(w, np.float32) + np.asarray(bb, np.float32)
        return y.reshape(B, -1, NH, D).transpose(0, 2, 1, 3)

    qs = split(q, p["wq"], p["bq"])
    ks = split(k, p["wk"], p["bk"])
    vs = split(v, p["wv"], p["bv"])
    scores = np.einsum("bhqd,bhkd->bhqk", qs, ks) / np.sqrt(D).astype(np.float32)
    aw = 1.0 / (1.0 + np.exp(-scores))
    aw = _softmax(aw if indicator == 0 else 1.0 - aw, axis=-1)
    o = np.einsum("bhqk,bhkd->bhqd", aw, vs).transpose(0, 2, 1, 3).reshape(B, -1, E)
    return o @ np.asarray(p["wo"], np.float32) + np.asarray(p["bo"], np.float32)


def _pool_all(x):
    return np.concatenate([x.max(0), x.min(0), x.mean(0)], 0).reshape(1, 96)


def _masked_pool(mask, x):
    mask = np.asarray(mask)
    mx = np.where(mask[:, :, None], x[None], -np.inf).max(1)
    mn = np.where(mask[:, :, None], x[None], np.inf).min(1)
    cnt = mask.sum(1, keepdims=True).astype(x.dtype)
    mean = (mask.astype(x.dtype) @ x) / cnt
    return np.concatenate([mx, mn, mean], axis=1)


def kernel(b_feature, h_feature, b_edge_list, h_edge_list, pos_mask, neg_mask,
           params):
    b_feature = np.asarray(b_feature, np.float32)
    h_feature = np.asarray(h_feature, np.float32)
    bei = np.asarray(b_edge_list).T
    hei = np.asarray(h_edge_list).T

    host_mm = lambda x, W: x @ W

    # Body graph: 3 GAT layers, x@W on the 8 NeuronCores.
    b = np.tanh(_gat(b_feature, bei, params["c1"], _device_mm))
    b = np.tanh(_gat(b, bei, params["c2"], _device_mm))
    b = np.tanh(_gat(b, bei, params["c3"], _device_mm))

    # Head graph: 64 nodes — negligible, replicated host compute.
    h = np.tanh(_gat(h_feature, hei, params["h1"], host_mm))
    h = np.tanh(_gat(h, hei, params["h2"], host_mm))

    pos_pooled = _masked_pool(pos_mask, b)
    neg_pooled = _masked_pool(neg_mask, b)
    head_pooled = _pool_all(h)
    pos_rep = _mha(head_pooled.reshape(1, 1, 96), pos_pooled[None],
                   pos_pooled[None], 0, params["attn"]).reshape(1, 96)
    neg_rep = _mha(head_pooled.reshape(1, 1, 96), neg_pooled[None],
                   neg_pooled[None], 1, params["attn"]).reshape(1, 96)
    global_rep = _pool_all(b)
    return head_pooled, pos_rep, neg_rep, global_rep


# revision 11
# speedup vs baseline: 2.2321x; 2.2321x over previous
"""Trainium2 Bass kernel for nn_DocLSTM_79018808312396 (GNN message passing).

Strategy: the dominant dense compute (the per-layer GAT feature transforms
x @ W for the 30000-node body graph, ~13.8 GFLOP total) runs as a Bass/Tile
SPMD kernel sharded node-parallel across the 8 NeuronCores (3840 rows/core,
one compiled NEFF reused for all three body layers via zero-padding to a
fixed [384-K, 256-N] shape). The irregular per-edge segment-softmax
bookkeeping, the tiny 64-node head graph, and the attention/pooling tail run
replicated on host (they are index-manipulation / sub-1% of the FLOPs).
"""

import numpy as np

N_BODY = 30000
M_PAD = 30720          # 30000 padded to 8 cores * 3840 (30 chunks of 128)
M_CORE = 3840
K_PAD = 384            # max fin (300) padded to 3 chunks of 128
N_PAD = 256            # max GAT_HEADS*fout (2*128)
N_CORES = 8

_NC_CACHE = {}


def _build_mm_nc():
    """out[m, n] = sum_k xT[k, m] * w[k, n]  (per core: [3840,384]@[384,256])"""
    import concourse.mybir as mybir
    import concourse.tile as tile
    from concourse import bacc

    nc = bacc.Bacc("TRN2", target_bir_lowering=False, debug=False,
                   num_devices=N_CORES)
    # Host pre-tiles x.T into contiguous [128,128] blocks: block (m, k) holds
    # xT[k*128:(k+1)*128, m*128:(m+1)*128] at rows (m*n_k + k)*128.
    xT = nc.dram_tensor(
        "xT", [(M_CORE // 128) * (K_PAD // 128) * 128, 128],
        mybir.dt.float32, kind="ExternalInput").ap()
    w = nc.dram_tensor("w", [K_PAD, N_PAD], mybir.dt.float32,
                       kind="ExternalInput").ap()
    out = nc.dram_tensor("out", [M_CORE, N_PAD], mybir.dt.float32,
                         kind="ExternalOutput").ap()

    n_m = M_CORE // 128   # 30 node chunks
    n_k = K_PAD // 128    # 3 contraction chunks

    with tile.TileContext(nc) as tc:
        with (
            tc.tile_pool(name="wpool", bufs=1) as wpool,
            tc.tile_pool(name="sb", bufs=4) as sb,
            tc.tile_pool(name="ob", bufs=3) as ob,
            tc.tile_pool(name="ps", bufs=2, space="PSUM") as ps,
        ):
            w_tiles = []
            for k in range(n_k):
                wt = wpool.tile([128, N_PAD], mybir.dt.float32, tag=f"w{k}")
                nc.gpsimd.dma_start(out=wt[:], in_=w[k * 128:(k + 1) * 128, :])
                w_tiles.append(wt)

            for m in range(n_m):
                psum = ps.tile([128, N_PAD], mybir.dt.float32)
                for k in range(n_k):
                    xt = sb.tile([128, 128], mybir.dt.float32, tag="xt")
                    blk = (m * n_k + k) * 128
                    nc.gpsimd.dma_start(out=xt[:], in_=xT[blk:blk + 128, :])
                    nc.tensor.matmul(
                        out=psum[:],
                        lhsT=xt[:],
                        rhs=w_tiles[k][:],
                        start=(k == 0),
                        stop=(k == n_k - 1),
                    )
                o = ob.tile([128, N_PAD], mybir.dt.float32, tag="o")
                nc.vector.tensor_copy(out=o[:], in_=psum[:])
                nc.sync.dma_start(out=out[m * 128:(m + 1) * 128, :], in_=o[:])
    nc.compile()
    return nc


def _device_mm(x, W):
    """x [30000, fin] @ W [fin, fout] -> [30000, fout] on 8 NeuronCores."""
    from concourse.bass_utils import run_bass_kernel_spmd

    fin, fout = W.shape
    if "nc" not in _NC_CACHE:
        _NC_CACHE["nc"] = _build_mm_nc()
    nc = _NC_CACHE["nc"]

    xTp = np.zeros((K_PAD, M_PAD), np.float32)
    xTp[:fin, :N_BODY] = np.asarray(x, np.float32).T
    wp = np.zeros((K_PAD, N_PAD), np.float32)
    wp[:fin, :fout] = np.asarray(W, np.float32)

    n_k = K_PAD // 128
    n_m = M_CORE // 128
    in_maps = []
    for c in range(N_CORES):
        sh = xTp[:, c * M_CORE:(c + 1) * M_CORE]          # [K_PAD, M_CORE]
        # -> [n_m*n_k*128, 128]: block (m, k) = sh[k*128:, m*128:]
        t = (sh.reshape(n_k, 128, n_m, 128)
               .transpose(2, 0, 1, 3)
               .reshape(n_m * n_k * 128, 128))
        in_maps.append({"xT": np.ascontiguousarray(t), "w": wp})
    res = run_bass_kernel_spmd(nc, in_maps, list(range(N_CORES))).results
    out = np.concatenate([np.asarray(res[c]["out"]) for c in range(N_CORES)],
                         axis=0)
    return out[:N_BODY, :fout]


def _leaky(x):
    return np.where(x >= 0, x, np.float32(0.2) * x)


_SORT_CACHE = {}


def _edge_sort(ei, N):
    """Sorted-by-dst edge structure (with self loops); same for all layers
    on one graph, so cache it."""
    ei = np.asarray(ei)
    key = (hash(ei.tobytes()), ei.shape, N)
    if key not in _SORT_CACHE:
        loop = np.arange(N, dtype=np.asarray(ei).dtype)
        src = np.concatenate([np.asarray(ei[0]), loop])
        dst = np.concatenate([np.asarray(ei[1]), loop])
        order = np.argsort(dst, kind="stable")
        ds_, ss = dst[order], src[order]
        starts = np.searchsorted(ds_, np.arange(N))  # every node has a loop
        _SORT_CACHE[key] = (order, ds_, ss, starts)
    return _SORT_CACHE[key]


def _gat(x, ei, p, xp_fn):
    """GATConv matching the reference; xp_fn computes x @ W."""
    x = np.asarray(x, np.float32)
    N = x.shape[0]
    W = np.asarray(p["W"], np.float32)
    a_src = np.asarray(p["a_src"], np.float32)
    a_dst = np.asarray(p["a_dst"], np.float32)
    b = np.asarray(p["b"], np.float32)
    H, F = a_src.shape

    xp = xp_fn(x, W).reshape(N, H, F)
    order, ds_, ss, starts = _edge_sort(ei, N)

    a_s = (xp * a_src).sum(-1)      # [N, H]
    a_d = (xp * a_dst).sum(-1)
    al = _leaky(a_s[ss] + a_d[ds_])   # [E, H], already in dst-sorted order

    m = np.maximum.reduceat(al, starts, axis=0)          # [N, H]
    e = np.exp(al - m[ds_])
    denom = np.add.reduceat(e, starts, axis=0)
    coef = e / np.maximum(denom[ds_], np.float32(1e-16))

    msg = coef[:, :, None].astype(np.float32) * xp[ss]   # [E, H, F]
    out = np.add.reduceat(msg, starts, axis=0)           # [N, H, F]
    return out.mean(1) + b


def _softmax(x, axis=-1):
    m = x.max(axis=axis, keepdims=True)
    e = np.exp(x - m)
    return e / e.sum(axis=axis, keepdims=True)


def _mha(q, k, v, indicator, p):
    B, E = q.shape[0], q.shape[-1]
    NH = 4
    D = E // NH

    def split(x, w, bb):
        y = x @ np.asarray(w, np.float32) + np.asarray(bb, np.float32)
        return y.reshape(B, -1, NH, D).transpose(0, 2, 1, 3)

    qs = split(q, p["wq"], p["bq"])
    ks = split(k, p["wk"], p["bk"])
    vs = split(v, p["wv"], p["bv"])
    scores = np.einsum("bhqd,bhkd->bhqk", qs, ks) / np.sqrt(D).astype(np.float32)
    aw = 1.0 / (1.0 + np.exp(-scores))
    aw = _softmax(aw if indicator == 0 else 1.0 - aw, axis=-1)
    o = np.einsum("bhqk,bhkd->bhqd", aw, vs).transpose(0, 2, 1, 3).reshape(B, -1, E)
    return o @ np.asarray(p["wo"], np.float32) + np.asarray(p["bo"], np.float32)


def _pool_all(x):
    return np.concatenate([x.max(0), x.min(0), x.mean(0)], 0).reshape(1, 96)


def _masked_pool(mask, x):
    mask = np.asarray(mask)
    mx = np.where(mask[:, :, None], x[None], -np.inf).max(1)
    mn = np.where(mask[:, :, None], x[None], np.inf).min(1)
    cnt = mask.sum(1, keepdims=True).astype(x.dtype)
    mean = (mask.astype(x.dtype) @ x) / cnt
    return np.concatenate([mx, mn, mean], axis=1)


def kernel(b_feature, h_feature, b_edge_list, h_edge_list, pos_mask, neg_mask,
           params):
    b_feature = np.asarray(b_feature, np.float32)
    h_feature = np.asarray(h_feature, np.float32)
    bei = np.asarray(b_edge_list).T
    hei = np.asarray(h_edge_list).T

    host_mm = lambda x, W: x @ W

    # Body graph: 3 GAT layers, x@W on the 8 NeuronCores.
    b = np.tanh(_gat(b_feature, bei, params["c1"], _device_mm))
    b = np.tanh(_gat(b, bei, params["c2"], _device_mm))
    b = np.tanh(_gat(b, bei, params["c3"], _device_mm))

    # Head graph: 64 nodes — negligible, replicated host compute.
    h = np.tanh(_gat(h_feature, hei, params["h1"], host_mm))
    h = np.tanh(_gat(h, hei, params["h2"], host_mm))

    pos_pooled = _masked_pool(pos_mask, b)
    neg_pooled = _masked_pool(neg_mask, b)
    head_pooled = _pool_all(h)
    pos_rep = _mha(head_pooled.reshape(1, 1, 96), pos_pooled[None],
                   pos_pooled[None], 0, params["attn"]).reshape(1, 96)
    neg_rep = _mha(head_pooled.reshape(1, 1, 96), neg_pooled[None],
                   neg_pooled[None], 1, params["attn"]).reshape(1, 96)
    global_rep = _pool_all(b)
    return head_pooled, pos_rep, neg_rep, global_rep


# revision 14
# speedup vs baseline: 2.7424x; 1.2286x over previous
"""Trainium2 Bass kernel for nn_DocLSTM_79018808312396 (GNN message passing).

Strategy: the dominant dense compute (the per-layer GAT feature transforms
x @ W for the 30000-node body graph, ~13.8 GFLOP total) runs as a Bass/Tile
SPMD kernel sharded node-parallel across the 8 NeuronCores (3840 rows/core,
one compiled NEFF reused for all three body layers via zero-padding to a
fixed [384-K, 256-N] shape). The irregular per-edge segment-softmax
bookkeeping, the tiny 64-node head graph, and the attention/pooling tail run
replicated on host (they are index-manipulation / sub-1% of the FLOPs).
"""

import numpy as np

N_BODY = 30000
M_PAD = 30720          # 30000 padded to 8 cores * 3840 (30 chunks of 128)
M_CORE = 3840
K_PAD = 384            # max fin (300) padded to 3 chunks of 128
N_PAD = 256            # max GAT_HEADS*fout (2*128)
N_CORES = 8

_NC_CACHE = {}


def _build_mm_nc(n_k, n_pad):
    """out[m, n] = sum_k xT[k, m] * w[k, n]  (per core:
    [3840, n_k*128] @ [n_k*128, n_pad])"""
    import concourse.mybir as mybir
    import concourse.tile as tile
    from concourse import bacc

    k_pad = n_k * 128
    nc = bacc.Bacc("TRN2", target_bir_lowering=False, debug=False,
                   num_devices=N_CORES)
    # Host pre-tiles x.T into contiguous [128,128] blocks: block (m, k) holds
    # xT[k*128:(k+1)*128, m*128:(m+1)*128] at rows (m*n_k + k)*128.
    xT = nc.dram_tensor(
        "xT", [(M_CORE // 128) * n_k * 128, 128],
        mybir.dt.float32, kind="ExternalInput").ap()
    w = nc.dram_tensor("w", [k_pad, n_pad], mybir.dt.float32,
                       kind="ExternalInput").ap()
    out = nc.dram_tensor("out", [M_CORE, n_pad], mybir.dt.float32,
                         kind="ExternalOutput").ap()

    n_m = M_CORE // 128   # 30 node chunks

    with tile.TileContext(nc) as tc:
        with (
            tc.tile_pool(name="wpool", bufs=1) as wpool,
            tc.tile_pool(name="sb", bufs=4) as sb,
            tc.tile_pool(name="ob", bufs=3) as ob,
            tc.tile_pool(name="ps", bufs=2, space="PSUM") as ps,
        ):
            w_tiles = []
            for k in range(n_k):
                wt = wpool.tile([128, n_pad], mybir.dt.float32, tag=f"w{k}")
                nc.gpsimd.dma_start(out=wt[:], in_=w[k * 128:(k + 1) * 128, :])
                w_tiles.append(wt)

            for m in range(n_m):
                psum = ps.tile([128, n_pad], mybir.dt.float32)
                for k in range(n_k):
                    xt = sb.tile([128, 128], mybir.dt.float32, tag="xt")
                    blk = (m * n_k + k) * 128
                    nc.gpsimd.dma_start(out=xt[:], in_=xT[blk:blk + 128, :])
                    nc.tensor.matmul(
                        out=psum[:],
                        lhsT=xt[:],
                        rhs=w_tiles[k][:],
                        start=(k == 0),
                        stop=(k == n_k - 1),
                    )
                o = ob.tile([128, n_pad], mybir.dt.float32, tag="o")
                nc.vector.tensor_copy(out=o[:], in_=psum[:])
                nc.sync.dma_start(out=out[m * 128:(m + 1) * 128, :], in_=o[:])
    nc.compile()
    return nc


def _device_mm(x, W):
    """x [30000, fin] @ W [fin, fout] -> [30000, fout] on 8 NeuronCores."""
    from concourse.bass_utils import run_bass_kernel_spmd

    fin, fout = W.shape
    n_k = (fin + 127) // 128
    k_pad = n_k * 128
    n_pad = ((fout + 127) // 128) * 128
    if (n_k, n_pad) not in _NC_CACHE:
        _NC_CACHE[(n_k, n_pad)] = _build_mm_nc(n_k, n_pad)
    nc = _NC_CACHE[(n_k, n_pad)]

    xTp = np.zeros((k_pad, M_PAD), np.float32)
    xTp[:fin, :N_BODY] = np.asarray(x, np.float32).T
    wp = np.zeros((k_pad, n_pad), np.float32)
    wp[:fin, :fout] = np.asarray(W, np.float32)

    n_m = M_CORE // 128
    in_maps = []
    for c in range(N_CORES):
        sh = xTp[:, c * M_CORE:(c + 1) * M_CORE]          # [K_PAD, M_CORE]
        # -> [n_m*n_k*128, 128]: block (m, k) = sh[k*128:, m*128:]
        t = (sh.reshape(n_k, 128, n_m, 128)
               .transpose(2, 0, 1, 3)
               .reshape(n_m * n_k * 128, 128))
        in_maps.append({"xT": np.ascontiguousarray(t), "w": wp})
    res = run_bass_kernel_spmd(nc, in_maps, list(range(N_CORES))).results
    out = np.concatenate([np.asarray(res[c]["out"]) for c in range(N_CORES)],
                         axis=0)
    return out[:N_BODY, :fout]


def _leaky(x):
    return np.where(x >= 0, x, np.float32(0.2) * x)


_SORT_CACHE = {}


def _edge_sort(ei, N):
    """Sorted-by-dst edge structure (with self loops); same for all layers
    on one graph, so cache it."""
    ei = np.asarray(ei)
    key = (hash(ei.tobytes()), ei.shape, N)
    if key not in _SORT_CACHE:
        loop = np.arange(N, dtype=np.asarray(ei).dtype)
        src = np.concatenate([np.asarray(ei[0]), loop])
        dst = np.concatenate([np.asarray(ei[1]), loop])
        order = np.argsort(dst, kind="stable")
        ds_, ss = dst[order], src[order]
        starts = np.searchsorted(ds_, np.arange(N))  # every node has a loop
        _SORT_CACHE[key] = (order, ds_, ss, starts)
    return _SORT_CACHE[key]


def _gat(x, ei, p, xp_fn):
    """GATConv matching the reference; xp_fn computes x @ W."""
    x = np.asarray(x, np.float32)
    N = x.shape[0]
    W = np.asarray(p["W"], np.float32)
    a_src = np.asarray(p["a_src"], np.float32)
    a_dst = np.asarray(p["a_dst"], np.float32)
    b = np.asarray(p["b"], np.float32)
    H, F = a_src.shape

    xp = xp_fn(x, W).reshape(N, H, F)
    order, ds_, ss, starts = _edge_sort(ei, N)

    a_s = (xp * a_src).sum(-1)      # [N, H]
    a_d = (xp * a_dst).sum(-1)
    al = _leaky(a_s[ss] + a_d[ds_])   # [E, H], already in dst-sorted order

    m = np.maximum.reduceat(al, starts, axis=0)          # [N, H]
    e = np.exp(al - m[ds_])
    denom = np.add.reduceat(e, starts, axis=0)
    coef = e / np.maximum(denom[ds_], np.float32(1e-16))

    msg = coef[:, :, None].astype(np.float32) * xp[ss]   # [E, H, F]
    out = np.add.reduceat(msg, starts, axis=0)           # [N, H, F]
    return out.mean(1) + b


def _softmax(x, axis=-1):
    m = x.max(axis=axis, keepdims=True)
    e = np.exp(x - m)
    return e / e.sum(axis=axis, keepdims=True)


def _mha(q, k, v, indicator, p):
    B, E = q.shape[0], q.shape[-1]
    NH = 4
    D = E // NH

    def split(x, w, bb):
        y = x @ np.asarray(w, np.float32) + np.asarray(bb, np.float32)
        return y.reshape(B, -1, NH, D).transpose(0, 2, 1, 3)

    qs = split(q, p["wq"], p["bq"])
    ks = split(k, p["wk"], p["bk"])
    vs = split(v, p["wv"], p["bv"])
    scores = np.einsum("bhqd,bhkd->bhqk", qs, ks) / np.sqrt(D).astype(np.float32)
    aw = 1.0 / (1.0 + np.exp(-scores))
    aw = _softmax(aw if indicator == 0 else 1.0 - aw, axis=-1)
    o = np.einsum("bhqk,bhkd->bhqd", aw, vs).transpose(0, 2, 1, 3).reshape(B, -1, E)
    return o @ np.asarray(p["wo"], np.float32) + np.asarray(p["bo"], np.float32)


def _pool_all(x):
    return np.concatenate([x.max(0), x.min(0), x.mean(0)], 0).reshape(1, 96)


def _masked_pool(mask, x):
    mask = np.asarray(mask)
    mx = np.where(mask[:, :, None], x[None], -np.inf).max(1)
    mn = np.where(mask[:, :, None], x[None], np.inf).min(1)
    cnt = mask.sum(1, keepdims=True).astype(x.dtype)
    mean = (mask.astype(x.dtype) @ x) / cnt
    return np.concatenate([mx, mn, mean], axis=1)


def kernel(b_feature, h_feature, b_edge_list, h_edge_list, pos_mask, neg_mask,
           params):
    b_feature = np.asarray(b_feature, np.float32)
    h_feature = np.asarray(h_feature, np.float32)
    bei = np.asarray(b_edge_list).T
    hei = np.asarray(h_edge_list).T

    host_mm = lambda x, W: x @ W

    # Body graph: 3 GAT layers, x@W on the 8 NeuronCores.
    b = np.tanh(_gat(b_feature, bei, params["c1"], _device_mm))
    b = np.tanh(_gat(b, bei, params["c2"], _device_mm))
    b = np.tanh(_gat(b, bei, params["c3"], _device_mm))

    # Head graph: 64 nodes — negligible, replicated host compute.
    h = np.tanh(_gat(h_feature, hei, params["h1"], host_mm))
    h = np.tanh(_gat(h, hei, params["h2"], host_mm))

    pos_pooled = _masked_pool(pos_mask, b)
    neg_pooled = _masked_pool(neg_mask, b)
    head_pooled = _pool_all(h)
    pos_rep = _mha(head_pooled.reshape(1, 1, 96), pos_pooled[None],
                   pos_pooled[None], 0, params["attn"]).reshape(1, 96)
    neg_rep = _mha(head_pooled.reshape(1, 1, 96), neg_pooled[None],
                   neg_pooled[None], 1, params["attn"]).reshape(1, 96)
    global_rep = _pool_all(b)
    return head_pooled, pos_rep, neg_rep, global_rep


# revision 15
# speedup vs baseline: 5.5357x; 2.0186x over previous
"""Trainium2 Bass kernel for nn_DocLSTM_79018808312396 (GNN message passing).

Strategy: the dominant dense compute (the per-layer GAT feature transforms
x @ W for the 30000-node body graph, ~13.8 GFLOP total) runs as a Bass/Tile
SPMD kernel sharded node-parallel across the 8 NeuronCores (3840 rows/core,
one compiled NEFF reused for all three body layers via zero-padding to a
fixed [384-K, 256-N] shape). The irregular per-edge segment-softmax
bookkeeping, the tiny 64-node head graph, and the attention/pooling tail run
replicated on host (they are index-manipulation / sub-1% of the FLOPs).
"""

import numpy as np

N_BODY = 30000
M_PAD = 30720          # 30000 padded to 8 cores * 3840 (30 chunks of 128)
M_CORE = 3840
K_PAD = 384            # max fin (300) padded to 3 chunks of 128
N_PAD = 256            # max GAT_HEADS*fout (2*128)
N_CORES = 8

_NC_CACHE = {}


def _build_mm_nc(n_k, n_pad):
    """out[m, n] = sum_k xT[k, m] * w[k, n]  (per core:
    [3840, n_k*128] @ [n_k*128, n_pad])"""
    import concourse.mybir as mybir
    import concourse.tile as tile
    from concourse import bacc

    k_pad = n_k * 128
    nc = bacc.Bacc("TRN2", target_bir_lowering=False, debug=False,
                   num_devices=N_CORES)
    # Host pre-tiles x.T into contiguous [128,128] blocks: block (m, k) holds
    # xT[k*128:(k+1)*128, m*128:(m+1)*128] at rows (m*n_k + k)*128.
    xT = nc.dram_tensor(
        "xT", [(M_CORE // 128) * n_k * 128, 128],
        mybir.dt.float32, kind="ExternalInput").ap()
    w = nc.dram_tensor("w", [k_pad, n_pad], mybir.dt.float32,
                       kind="ExternalInput").ap()
    out = nc.dram_tensor("out", [M_CORE, n_pad], mybir.dt.float32,
                         kind="ExternalOutput").ap()

    n_m = M_CORE // 128   # 30 node chunks

    with tile.TileContext(nc) as tc:
        with (
            tc.tile_pool(name="wpool", bufs=1) as wpool,
            tc.tile_pool(name="sb", bufs=4) as sb,
            tc.tile_pool(name="ob", bufs=3) as ob,
            tc.tile_pool(name="ps", bufs=2, space="PSUM") as ps,
        ):
            w_tiles = []
            for k in range(n_k):
                wt = wpool.tile([128, n_pad], mybir.dt.float32, tag=f"w{k}")
                nc.gpsimd.dma_start(out=wt[:], in_=w[k * 128:(k + 1) * 128, :])
                w_tiles.append(wt)

            for m in range(n_m):
                psum = ps.tile([128, n_pad], mybir.dt.float32)
                for k in range(n_k):
                    xt = sb.tile([128, 128], mybir.dt.float32, tag="xt")
                    blk = (m * n_k + k) * 128
                    nc.gpsimd.dma_start(out=xt[:], in_=xT[blk:blk + 128, :])
                    nc.tensor.matmul(
                        out=psum[:],
                        lhsT=xt[:],
                        rhs=w_tiles[k][:],
                        start=(k == 0),
                        stop=(k == n_k - 1),
                    )
                o = ob.tile([128, n_pad], mybir.dt.float32, tag="o")
                nc.vector.tensor_copy(out=o[:], in_=psum[:])
                nc.sync.dma_start(out=out[m * 128:(m + 1) * 128, :], in_=o[:])
    nc.compile()
    return nc


def _device_mm(x, W):
    """x [30000, fin] @ W [fin, fout] -> [30000, fout] on 8 NeuronCores."""
    from concourse.bass_utils import run_bass_kernel_spmd

    fin, fout = W.shape
    n_k = (fin + 127) // 128
    k_pad = n_k * 128
    n_pad = ((fout + 127) // 128) * 128
    if (n_k, n_pad) not in _NC_CACHE:
        _NC_CACHE[(n_k, n_pad)] = _build_mm_nc(n_k, n_pad)
    nc = _NC_CACHE[(n_k, n_pad)]

    xTp = np.zeros((k_pad, M_PAD), np.float32)
    xTp[:fin, :N_BODY] = np.asarray(x, np.float32).T
    wp = np.zeros((k_pad, n_pad), np.float32)
    wp[:fin, :fout] = np.asarray(W, np.float32)

    n_m = M_CORE // 128
    in_maps = []
    for c in range(N_CORES):
        sh = xTp[:, c * M_CORE:(c + 1) * M_CORE]          # [K_PAD, M_CORE]
        # -> [n_m*n_k*128, 128]: block (m, k) = sh[k*128:, m*128:]
        t = (sh.reshape(n_k, 128, n_m, 128)
               .transpose(2, 0, 1, 3)
               .reshape(n_m * n_k * 128, 128))
        in_maps.append({"xT": np.ascontiguousarray(t), "w": wp})
    res = run_bass_kernel_spmd(nc, in_maps, list(range(N_CORES))).results
    out = np.concatenate([np.asarray(res[c]["out"]) for c in range(N_CORES)],
                         axis=0)
    return out[:N_BODY, :fout]


def _leaky(x):
    return np.where(x >= 0, x, np.float32(0.2) * x)


_SORT_CACHE = {}


def _edge_sort(ei, N):
    """Sorted-by-dst edge structure (with self loops); same for all layers
    on one graph, so cache it."""
    ei = np.asarray(ei)
    key = (hash(ei.tobytes()), ei.shape, N)
    if key not in _SORT_CACHE:
        loop = np.arange(N, dtype=np.asarray(ei).dtype)
        src = np.concatenate([np.asarray(ei[0]), loop])
        dst = np.concatenate([np.asarray(ei[1]), loop])
        order = np.argsort(dst, kind="stable")
        ds_, ss = dst[order], src[order]
        starts = np.searchsorted(ds_, np.arange(N))  # every node has a loop
        _SORT_CACHE[key] = (order, ds_, ss, starts)
    return _SORT_CACHE[key]


def _gat(x, ei, p, xp_fn):
    """GATConv matching the reference; xp_fn computes x @ W."""
    x = np.asarray(x, np.float32)
    N = x.shape[0]
    W = np.asarray(p["W"], np.float32)
    a_src = np.asarray(p["a_src"], np.float32)
    a_dst = np.asarray(p["a_dst"], np.float32)
    b = np.asarray(p["b"], np.float32)
    H, F = a_src.shape

    xp = xp_fn(x, W).reshape(N, H, F)
    order, ds_, ss, starts = _edge_sort(ei, N)

    a_s = (xp * a_src).sum(-1)      # [N, H]
    a_d = (xp * a_dst).sum(-1)
    al = _leaky(a_s[ss] + a_d[ds_])   # [E, H], already in dst-sorted order

    m = np.maximum.reduceat(al, starts, axis=0)          # [N, H]
    e = np.exp(al - m[ds_])
    denom = np.add.reduceat(e, starts, axis=0)
    coef = e / np.maximum(denom[ds_], np.float32(1e-16))

    # out[d] = sum_{e: dst=d} coef_e * xp[src_e]  — per-head CSR matmul with
    # the (cached) sorted-by-dst structure; only the data vector changes.
    from scipy.sparse import csr_matrix
    indptr = np.concatenate([starts, [len(ss)]]).astype(np.int64)
    acc = np.zeros((N, F), np.float32)
    for h in range(H):
        A = csr_matrix((coef[:, h].astype(np.float32), ss, indptr),
                       shape=(N, N))
        acc += A @ np.ascontiguousarray(xp[:, h, :])
    return acc * np.float32(1.0 / H) + b


def _softmax(x, axis=-1):
    m = x.max(axis=axis, keepdims=True)
    e = np.exp(x - m)
    return e / e.sum(axis=axis, keepdims=True)


def _mha(q, k, v, indicator, p):
    B, E = q.shape[0], q.shape[-1]
    NH = 4
    D = E // NH

    def split(x, w, bb):
        y = x @ np.asarray(w, np.float32) + np.asarray(bb, np.float32)
        return y.reshape(B, -1, NH, D).transpose(0, 2, 1, 3)

    qs = split(q, p["wq"], p["bq"])
    ks = split(k, p["wk"], p["bk"])
    vs = split(v, p["wv"], p["bv"])
    scores = np.einsum("bhqd,bhkd->bhqk", qs, ks) / np.sqrt(D).astype(np.float32)
    aw = 1.0 / (1.0 + np.exp(-scores))
    aw = _softmax(aw if indicator == 0 else 1.0 - aw, axis=-1)
    o = np.einsum("bhqk,bhkd->bhqd", aw, vs).transpose(0, 2, 1, 3).reshape(B, -1, E)
    return o @ np.asarray(p["wo"], np.float32) + np.asarray(p["bo"], np.float32)


def _pool_all(x):
    return np.concatenate([x.max(0), x.min(0), x.mean(0)], 0).reshape(1, 96)


def _masked_pool(mask, x):
    mask = np.asarray(mask)
    mx = np.where(mask[:, :, None], x[None], -np.inf).max(1)
    mn = np.where(mask[:, :, None], x[None], np.inf).min(1)
    cnt = mask.sum(1, keepdims=True).astype(x.dtype)
    mean = (mask.astype(x.dtype) @ x) / cnt
    return np.concatenate([mx, mn, mean], axis=1)


def kernel(b_feature, h_feature, b_edge_list, h_edge_list, pos_mask, neg_mask,
           params):
    b_feature = np.asarray(b_feature, np.float32)
    h_feature = np.asarray(h_feature, np.float32)
    bei = np.asarray(b_edge_list).T
    hei = np.asarray(h_edge_list).T

    host_mm = lambda x, W: x @ W

    # Body graph: 3 GAT layers, x@W on the 8 NeuronCores.
    b = np.tanh(_gat(b_feature, bei, params["c1"], _device_mm))
    b = np.tanh(_gat(b, bei, params["c2"], _device_mm))
    b = np.tanh(_gat(b, bei, params["c3"], _device_mm))

    # Head graph: 64 nodes — negligible, replicated host compute.
    h = np.tanh(_gat(h_feature, hei, params["h1"], host_mm))
    h = np.tanh(_gat(h, hei, params["h2"], host_mm))

    pos_pooled = _masked_pool(pos_mask, b)
    neg_pooled = _masked_pool(neg_mask, b)
    head_pooled = _pool_all(h)
    pos_rep = _mha(head_pooled.reshape(1, 1, 96), pos_pooled[None],
                   pos_pooled[None], 0, params["attn"]).reshape(1, 96)
    neg_rep = _mha(head_pooled.reshape(1, 1, 96), neg_pooled[None],
                   neg_pooled[None], 1, params["attn"]).reshape(1, 96)
    global_rep = _pool_all(b)
    return head_pooled, pos_rep, neg_rep, global_rep


# revision 18
# speedup vs baseline: 5.7249x; 1.0342x over previous
"""Trainium2 Bass kernel for nn_DocLSTM_79018808312396 (GNN message passing).

Strategy: the dominant dense compute (the per-layer GAT feature transforms
x @ W for the 30000-node body graph, ~13.8 GFLOP total) runs as a Bass/Tile
SPMD kernel sharded node-parallel across the 8 NeuronCores (3840 rows/core,
one compiled NEFF reused for all three body layers via zero-padding to a
fixed [384-K, 256-N] shape). The irregular per-edge segment-softmax
bookkeeping, the tiny 64-node head graph, and the attention/pooling tail run
replicated on host (they are index-manipulation / sub-1% of the FLOPs).
"""

import numpy as np

N_BODY = 30000
M_PAD = 30720          # 30000 padded to 8 cores * 3840 (30 chunks of 128)
M_CORE = 3840
K_PAD = 384            # max fin (300) padded to 3 chunks of 128
N_PAD = 256            # max GAT_HEADS*fout (2*128)
N_CORES = 8

_NC_CACHE = {}


def _build_mm_nc(fin, n_pad):
    """out[m, n] = sum_k xT[k, m] * w[k, n]  (per core:
    [3840, fin] @ [fin, n_pad]); fin chunked into <=128-row K blocks with no
    zero padding."""
    import concourse.mybir as mybir
    import concourse.tile as tile
    from concourse import bacc

    k_sizes = [128] * (fin // 128) + ([fin % 128] if fin % 128 else [])
    k_offs = [sum(k_sizes[:i]) for i in range(len(k_sizes))]
    nc = bacc.Bacc("TRN2", target_bir_lowering=False, debug=False,
                   num_devices=N_CORES)
    # Host layout: [n_m, fin, 128] flattened — node-chunk m's transposed
    # slab xT[:, m*128:(m+1)*128] at rows [m*fin, (m+1)*fin).
    xT = nc.dram_tensor(
        "xT", [(M_CORE // 128) * fin, 128],
        mybir.dt.float32, kind="ExternalInput").ap()
    w = nc.dram_tensor("w", [fin, n_pad], mybir.dt.float32,
                       kind="ExternalInput").ap()
    out = nc.dram_tensor("out", [M_CORE, n_pad], mybir.dt.float32,
                         kind="ExternalOutput").ap()

    n_m = M_CORE // 128   # 30 node chunks
    n_k = len(k_sizes)

    with tile.TileContext(nc) as tc:
        with (
            tc.tile_pool(name="wpool", bufs=1) as wpool,
            tc.tile_pool(name="sb", bufs=4) as sb,
            tc.tile_pool(name="ob", bufs=3) as ob,
            tc.tile_pool(name="ps", bufs=2, space="PSUM") as ps,
        ):
            w_tiles = []
            for k, (ko, ks) in enumerate(zip(k_offs, k_sizes)):
                wt = wpool.tile([128, n_pad], mybir.dt.float32, tag=f"w{k}")
                nc.gpsimd.dma_start(out=wt[:ks], in_=w[ko:ko + ks, :])
                w_tiles.append(wt)

            for m in range(n_m):
                psum = ps.tile([128, n_pad], mybir.dt.float32)
                for k, (ko, ks) in enumerate(zip(k_offs, k_sizes)):
                    xt = sb.tile([128, 128], mybir.dt.float32, tag="xt")
                    blk = m * fin + ko
                    nc.gpsimd.dma_start(out=xt[:ks], in_=xT[blk:blk + ks, :])
                    nc.tensor.matmul(
                        out=psum[:],
                        lhsT=xt[:ks],
                        rhs=w_tiles[k][:ks],
                        start=(k == 0),
                        stop=(k == n_k - 1),
                    )
                o = ob.tile([128, n_pad], mybir.dt.float32, tag="o")
                nc.vector.tensor_copy(out=o[:], in_=psum[:])
                nc.sync.dma_start(out=out[m * 128:(m + 1) * 128, :], in_=o[:])
    nc.compile()
    return nc


def _device_mm(x, W):
    """x [30000, fin] @ W [fin, fout] -> [30000, fout] on 8 NeuronCores."""
    from concourse.bass_utils import run_bass_kernel_spmd

    fin, fout = W.shape
    n_pad = ((fout + 127) // 128) * 128
    if (fin, n_pad) not in _NC_CACHE:
        _NC_CACHE[(fin, n_pad)] = _build_mm_nc(fin, n_pad)
    nc = _NC_CACHE[(fin, n_pad)]

    xTp = np.zeros((fin, M_PAD), np.float32)
    xTp[:, :N_BODY] = np.asarray(x, np.float32).T
    wp = np.zeros((fin, n_pad), np.float32)
    wp[:, :fout] = np.asarray(W, np.float32)

    n_m = M_CORE // 128
    in_maps = []
    for c in range(N_CORES):
        sh = xTp[:, c * M_CORE:(c + 1) * M_CORE]          # [fin, M_CORE]
        # -> [n_m*fin, 128]: node-chunk m's slab sh[:, m*128:] at m*fin
        t = (sh.reshape(fin, n_m, 128)
               .transpose(1, 0, 2)
               .reshape(n_m * fin, 128))
        in_maps.append({"xT": np.ascontiguousarray(t), "w": wp})
    res = run_bass_kernel_spmd(nc, in_maps, list(range(N_CORES))).results
    out = np.concatenate([np.asarray(res[c]["out"]) for c in range(N_CORES)],
                         axis=0)
    return out[:N_BODY, :fout]


def _leaky(x):
    return np.where(x >= 0, x, np.float32(0.2) * x)


_SORT_CACHE = {}


def _edge_sort(ei, N):
    """Sorted-by-dst edge structure (with self loops); same for all layers
    on one graph, so cache it."""
    ei = np.asarray(ei)
    key = (hash(ei.tobytes()), ei.shape, N)
    if key not in _SORT_CACHE:
        loop = np.arange(N, dtype=np.asarray(ei).dtype)
        src = np.concatenate([np.asarray(ei[0]), loop])
        dst = np.concatenate([np.asarray(ei[1]), loop])
        order = np.argsort(dst, kind="stable")
        ds_, ss = dst[order], src[order]
        starts = np.searchsorted(ds_, np.arange(N))  # every node has a loop
        _SORT_CACHE[key] = (order, ds_, ss, starts)
    return _SORT_CACHE[key]


def _gat(x, ei, p, xp_fn):
    """GATConv matching the reference; xp_fn computes x @ W."""
    x = np.asarray(x, np.float32)
    N = x.shape[0]
    W = np.asarray(p["W"], np.float32)
    a_src = np.asarray(p["a_src"], np.float32)
    a_dst = np.asarray(p["a_dst"], np.float32)
    b = np.asarray(p["b"], np.float32)
    H, F = a_src.shape

    xp = xp_fn(x, W).reshape(N, H, F)
    order, ds_, ss, starts = _edge_sort(ei, N)

    a_s = (xp * a_src).sum(-1)      # [N, H]
    a_d = (xp * a_dst).sum(-1)
    al = _leaky(a_s[ss] + a_d[ds_])   # [E, H], already in dst-sorted order

    m = np.maximum.reduceat(al, starts, axis=0)          # [N, H]
    e = np.exp(al - m[ds_])
    denom = np.add.reduceat(e, starts, axis=0)
    coef = e / np.maximum(denom[ds_], np.float32(1e-16))

    # out[d] = sum_{e: dst=d} coef_e * xp[src_e]  — per-head CSR matmul with
    # the (cached) sorted-by-dst structure; only the data vector changes.
    from scipy.sparse import csr_matrix
    indptr = np.concatenate([starts, [len(ss)]]).astype(np.int64)
    acc = np.zeros((N, F), np.float32)
    for h in range(H):
        A = csr_matrix((coef[:, h].astype(np.float32), ss, indptr),
                       shape=(N, N))
        acc += A @ np.ascontiguousarray(xp[:, h, :])
    return acc * np.float32(1.0 / H) + b


def _softmax(x, axis=-1):
    m = x.max(axis=axis, keepdims=True)
    e = np.exp(x - m)
    return e / e.sum(axis=axis, keepdims=True)


def _mha(q, k, v, indicator, p):
    B, E = q.shape[0], q.shape[-1]
    NH = 4
    D = E // NH

    def split(x, w, bb):
        y = x @ np.asarray(w, np.float32) + np.asarray(bb, np.float32)
        return y.reshape(B, -1, NH, D).transpose(0, 2, 1, 3)

    qs = split(q, p["wq"], p["bq"])
    ks = split(k, p["wk"], p["bk"])
    vs = split(v, p["wv"], p["bv"])
    scores = np.einsum("bhqd,bhkd->bhqk", qs, ks) / np.sqrt(D).astype(np.float32)
    aw = 1.0 / (1.0 + np.exp(-scores))
    aw = _softmax(aw if indicator == 0 else 1.0 - aw, axis=-1)
    o = np.einsum("bhqk,bhkd->bhqd", aw, vs).transpose(0, 2, 1, 3).reshape(B, -1, E)
    return o @ np.asarray(p["wo"], np.float32) + np.asarray(p["bo"], np.float32)


def _pool_all(x):
    return np.concatenate([x.max(0), x.min(0), x.mean(0)], 0).reshape(1, 96)


def _masked_pool(mask, x):
    mask = np.asarray(mask)
    mx = np.where(mask[:, :, None], x[None], -np.inf).max(1)
    mn = np.where(mask[:, :, None], x[None], np.inf).min(1)
    cnt = mask.sum(1, keepdims=True).astype(x.dtype)
    mean = (mask.astype(x.dtype) @ x) / cnt
    return np.concatenate([mx, mn, mean], axis=1)


def kernel(b_feature, h_feature, b_edge_list, h_edge_list, pos_mask, neg_mask,
           params):
    b_feature = np.asarray(b_feature, np.float32)
    h_feature = np.asarray(h_feature, np.float32)
    bei = np.asarray(b_edge_list).T
    hei = np.asarray(h_edge_list).T

    host_mm = lambda x, W: x @ W

    # Body graph: 3 GAT layers, x@W on the 8 NeuronCores.
    b = np.tanh(_gat(b_feature, bei, params["c1"], _device_mm))
    b = np.tanh(_gat(b, bei, params["c2"], _device_mm))
    b = np.tanh(_gat(b, bei, params["c3"], _device_mm))

    # Head graph: 64 nodes — negligible, replicated host compute.
    h = np.tanh(_gat(h_feature, hei, params["h1"], host_mm))
    h = np.tanh(_gat(h, hei, params["h2"], host_mm))

    pos_pooled = _masked_pool(pos_mask, b)
    neg_pooled = _masked_pool(neg_mask, b)
    head_pooled = _pool_all(h)
    pos_rep = _mha(head_pooled.reshape(1, 1, 96), pos_pooled[None],
                   pos_pooled[None], 0, params["attn"]).reshape(1, 96)
    neg_rep = _mha(head_pooled.reshape(1, 1, 96), neg_pooled[None],
                   neg_pooled[None], 1, params["attn"]).reshape(1, 96)
    global_rep = _pool_all(b)
    return head_pooled, pos_rep, neg_rep, global_rep


# revision 19
# speedup vs baseline: 5.9225x; 1.0345x over previous
"""Trainium2 Bass kernel for nn_DocLSTM_79018808312396 (GNN message passing).

Strategy: the dominant dense compute (the per-layer GAT feature transforms
x @ W for the 30000-node body graph, ~13.8 GFLOP total) runs as a Bass/Tile
SPMD kernel sharded node-parallel across the 8 NeuronCores (3840 rows/core,
one compiled NEFF reused for all three body layers via zero-padding to a
fixed [384-K, 256-N] shape). The irregular per-edge segment-softmax
bookkeeping, the tiny 64-node head graph, and the attention/pooling tail run
replicated on host (they are index-manipulation / sub-1% of the FLOPs).
"""

import numpy as np

N_BODY = 30000
M_PAD = 30720          # 30000 padded to 8 cores * 3840 (30 chunks of 128)
M_CORE = 3840
K_PAD = 384            # max fin (300) padded to 3 chunks of 128
N_PAD = 256            # max GAT_HEADS*fout (2*128)
N_CORES = 8

_NC_CACHE = {}


def _build_mm_nc(fin, n_pad):
    """out[m, n] = sum_k xT[k, m] * w[k, n]  (per core:
    [3840, fin] @ [fin, n_pad]); fin chunked into <=128-row K blocks with no
    zero padding."""
    import concourse.mybir as mybir
    import concourse.tile as tile
    from concourse import bacc

    k_sizes = [128] * (fin // 128) + ([fin % 128] if fin % 128 else [])
    k_offs = [sum(k_sizes[:i]) for i in range(len(k_sizes))]
    nc = bacc.Bacc("TRN2", target_bir_lowering=False, debug=False,
                   num_devices=N_CORES)
    # Host layout: [n_m, fin, 128] flattened — node-chunk m's transposed
    # slab xT[:, m*128:(m+1)*128] at rows [m*fin, (m+1)*fin).
    xT = nc.dram_tensor(
        "xT", [(M_CORE // 128) * fin, 128],
        mybir.dt.float32, kind="ExternalInput").ap()
    w = nc.dram_tensor("w", [fin, n_pad], mybir.dt.float32,
                       kind="ExternalInput").ap()
    out = nc.dram_tensor("out", [M_CORE, n_pad], mybir.dt.float32,
                         kind="ExternalOutput").ap()

    n_m = M_CORE // 128   # 30 node chunks
    n_k = len(k_sizes)

    with tile.TileContext(nc) as tc:
        with (
            tc.tile_pool(name="wpool", bufs=1) as wpool,
            tc.tile_pool(name="sb", bufs=4) as sb,
            tc.tile_pool(name="ob", bufs=3) as ob,
            tc.tile_pool(name="ps", bufs=2, space="PSUM") as ps,
        ):
            w_tiles = []
            for k, (ko, ks) in enumerate(zip(k_offs, k_sizes)):
                wt = wpool.tile([128, n_pad], mybir.dt.float32, tag=f"w{k}")
                nc.gpsimd.dma_start(out=wt[:ks], in_=w[ko:ko + ks, :])
                w_tiles.append(wt)

            for m in range(n_m):
                psum = ps.tile([128, n_pad], mybir.dt.float32)
                for k, (ko, ks) in enumerate(zip(k_offs, k_sizes)):
                    xt = sb.tile([128, 128], mybir.dt.float32, tag="xt")
                    blk = m * fin + ko
                    nc.gpsimd.dma_start(out=xt[:ks], in_=xT[blk:blk + ks, :])
                    nc.tensor.matmul(
                        out=psum[:],
                        lhsT=xt[:ks],
                        rhs=w_tiles[k][:ks],
                        start=(k == 0),
                        stop=(k == n_k - 1),
                    )
                o = ob.tile([128, n_pad], mybir.dt.float32, tag="o")
                nc.vector.tensor_copy(out=o[:], in_=psum[:])
                nc.sync.dma_start(out=out[m * 128:(m + 1) * 128, :], in_=o[:])
    nc.compile()
    return nc


def _device_mm(x, W):
    """x [30000, fin] @ W [fin, fout] -> [30000, fout] on 8 NeuronCores."""
    from concourse.bass_utils import run_bass_kernel_spmd

    fin, fout = W.shape
    n_pad = fout   # matmul free axis needs no 128-alignment
    if (fin, n_pad) not in _NC_CACHE:
        _NC_CACHE[(fin, n_pad)] = _build_mm_nc(fin, n_pad)
    nc = _NC_CACHE[(fin, n_pad)]

    xTp = np.zeros((fin, M_PAD), np.float32)
    xTp[:, :N_BODY] = np.asarray(x, np.float32).T
    wp = np.zeros((fin, n_pad), np.float32)
    wp[:, :fout] = np.asarray(W, np.float32)

    n_m = M_CORE // 128
    in_maps = []
    for c in range(N_CORES):
        sh = xTp[:, c * M_CORE:(c + 1) * M_CORE]          # [fin, M_CORE]
        # -> [n_m*fin, 128]: node-chunk m's slab sh[:, m*128:] at m*fin
        t = (sh.reshape(fin, n_m, 128)
               .transpose(1, 0, 2)
               .reshape(n_m * fin, 128))
        in_maps.append({"xT": np.ascontiguousarray(t), "w": wp})
    res = run_bass_kernel_spmd(nc, in_maps, list(range(N_CORES))).results
    out = np.concatenate([np.asarray(res[c]["out"]) for c in range(N_CORES)],
                         axis=0)
    return out[:N_BODY, :fout]


def _leaky(x):
    return np.where(x >= 0, x, np.float32(0.2) * x)


_SORT_CACHE = {}


def _edge_sort(ei, N):
    """Sorted-by-dst edge structure (with self loops); same for all layers
    on one graph, so cache it."""
    ei = np.asarray(ei)
    key = (hash(ei.tobytes()), ei.shape, N)
    if key not in _SORT_CACHE:
        loop = np.arange(N, dtype=np.asarray(ei).dtype)
        src = np.concatenate([np.asarray(ei[0]), loop])
        dst = np.concatenate([np.asarray(ei[1]), loop])
        order = np.argsort(dst, kind="stable")
        ds_, ss = dst[order], src[order]
        starts = np.searchsorted(ds_, np.arange(N))  # every node has a loop
        _SORT_CACHE[key] = (order, ds_, ss, starts)
    return _SORT_CACHE[key]


def _gat(x, ei, p, xp_fn):
    """GATConv matching the reference; xp_fn computes x @ W."""
    x = np.asarray(x, np.float32)
    N = x.shape[0]
    W = np.asarray(p["W"], np.float32)
    a_src = np.asarray(p["a_src"], np.float32)
    a_dst = np.asarray(p["a_dst"], np.float32)
    b = np.asarray(p["b"], np.float32)
    H, F = a_src.shape

    xp = xp_fn(x, W).reshape(N, H, F)
    order, ds_, ss, starts = _edge_sort(ei, N)

    a_s = (xp * a_src).sum(-1)      # [N, H]
    a_d = (xp * a_dst).sum(-1)
    al = _leaky(a_s[ss] + a_d[ds_])   # [E, H], already in dst-sorted order

    m = np.maximum.reduceat(al, starts, axis=0)          # [N, H]
    e = np.exp(al - m[ds_])
    denom = np.add.reduceat(e, starts, axis=0)
    coef = e / np.maximum(denom[ds_], np.float32(1e-16))

    # out[d] = sum_{e: dst=d} coef_e * xp[src_e]  — per-head CSR matmul with
    # the (cached) sorted-by-dst structure; only the data vector changes.
    from scipy.sparse import csr_matrix
    indptr = np.concatenate([starts, [len(ss)]]).astype(np.int64)
    acc = np.zeros((N, F), np.float32)
    for h in range(H):
        A = csr_matrix((coef[:, h].astype(np.float32), ss, indptr),
                       shape=(N, N))
        acc += A @ np.ascontiguousarray(xp[:, h, :])
    return acc * np.float32(1.0 / H) + b


def _softmax(x, axis=-1):
    m = x.max(axis=axis, keepdims=True)
    e = np.exp(x - m)
    return e / e.sum(axis=axis, keepdims=True)


def _mha(q, k, v, indicator, p):
    B, E = q.shape[0], q.shape[-1]
    NH = 4
    D = E // NH

    def split(x, w, bb):
        y = x @ np.asarray(w, np.float32) + np.asarray(bb, np.float32)
        return y.reshape(B, -1, NH, D).transpose(0, 2, 1, 3)

    qs = split(q, p["wq"], p["bq"])
    ks = split(k, p["wk"], p["bk"])
    vs = split(v, p["wv"], p["bv"])
    scores = np.einsum("bhqd,bhkd->bhqk", qs, ks) / np.sqrt(D).astype(np.float32)
    aw = 1.0 / (1.0 + np.exp(-scores))
    aw = _softmax(aw if indicator == 0 else 1.0 - aw, axis=-1)
    o = np.einsum("bhqk,bhkd->bhqd", aw, vs).transpose(0, 2, 1, 3).reshape(B, -1, E)
    return o @ np.asarray(p["wo"], np.float32) + np.asarray(p["bo"], np.float32)


def _pool_all(x):
    return np.concatenate([x.max(0), x.min(0), x.mean(0)], 0).reshape(1, 96)


def _masked_pool(mask, x):
    mask = np.asarray(mask)
    mx = np.where(mask[:, :, None], x[None], -np.inf).max(1)
    mn = np.where(mask[:, :, None], x[None], np.inf).min(1)
    cnt = mask.sum(1, keepdims=True).astype(x.dtype)
    mean = (mask.astype(x.dtype) @ x) / cnt
    return np.concatenate([mx, mn, mean], axis=1)


def kernel(b_feature, h_feature, b_edge_list, h_edge_list, pos_mask, neg_mask,
           params):
    b_feature = np.asarray(b_feature, np.float32)
    h_feature = np.asarray(h_feature, np.float32)
    bei = np.asarray(b_edge_list).T
    hei = np.asarray(h_edge_list).T

    host_mm = lambda x, W: x @ W

    # Body graph: 3 GAT layers, x@W on the 8 NeuronCores.
    b = np.tanh(_gat(b_feature, bei, params["c1"], _device_mm))
    b = np.tanh(_gat(b, bei, params["c2"], _device_mm))
    b = np.tanh(_gat(b, bei, params["c3"], _device_mm))

    # Head graph: 64 nodes — negligible, replicated host compute.
    h = np.tanh(_gat(h_feature, hei, params["h1"], host_mm))
    h = np.tanh(_gat(h, hei, params["h2"], host_mm))

    pos_pooled = _masked_pool(pos_mask, b)
    neg_pooled = _masked_pool(neg_mask, b)
    head_pooled = _pool_all(h)
    pos_rep = _mha(head_pooled.reshape(1, 1, 96), pos_pooled[None],
                   pos_pooled[None], 0, params["attn"]).reshape(1, 96)
    neg_rep = _mha(head_pooled.reshape(1, 1, 96), neg_pooled[None],
                   neg_pooled[None], 1, params["attn"]).reshape(1, 96)
    global_rep = _pool_all(b)
    return head_pooled, pos_rep, neg_rep, global_rep
